# revision 1
# baseline (speedup 1.0000x reference)
"""DepthToSpace (block_size=2, CRD layout) Trainium2 Bass kernel.

x: [16, 256, 128, 128] f32  ->  out: [16, 64, 256, 256] f32
out[b, dd, 2h+i, 2w+k] = x[b, (2i+k)*64 + dd, h, w]

Sharding: batch dim split across 8 NeuronCores (2 examples per core),
no communication. Per core the kernel is a pure reshuffle:
  - partition axis p = (b_local, dd) = 2*64 = 128 partitions
  - per 16-row tile: eight 512 KiB HBM->SBUF read DMAs with clean 2-dim
    access patterns ([[16384,64],[1,2048]] - multi-dim free APs are ~2x
    slower through HWDGE descriptor generation), DVE strided copies do
    the 2x2 pixel-shuffle interleave in SBUF, one 4 MiB SBUF->HBM write
    DMA with fully contiguous 32 KiB runs per partition.
  - DMA traffic is split BY DIRECTION across the queues: all reads are
    round-robined over the sync(SP) and gpsimd rings, all writes go on
    the scalar(ACT) ring.  Both directions then stream concurrently,
    which measures ~3x faster than the phase-alternating schedule
    (writes fill read-latency gaps; no ring carries mixed traffic and
    no ring is oversubscribed).
  - no explicit cross-phase sync edges: the tile pools (3 input tiles,
    3 output tiles) bound the pipeline depth; Tile's WAR/RAW tracking
    provides the pacing.
"""

import numpy as np

import concourse.bass as bass  # noqa: F401  (registers AP machinery)
import concourse.tile as tile
from concourse import bacc, bass_utils, mybir

# Problem shape (hardcoded per spec).
B, C, H, W = 16, 256, 128, 128
NCORES = 8
BL = B // NCORES  # local batch per core = 2
D = C // 4        # out channels = 64
HT = 16           # input rows per tile
NT = H // HT      # tiles per core = 8

_cached_nc = None


def _build(reps: int = 1):
    nc = bacc.Bacc(
        "TRN2",
        target_bir_lowering=False,
        debug=False,
        num_devices=NCORES,
    )
    x = nc.dram_tensor(
        "x", [BL, C, H, W], mybir.dt.float32, kind="ExternalInput"
    ).ap()
    out = nc.dram_tensor(
        "out", [BL, D, 2 * H, 2 * W], mybir.dt.float32, kind="ExternalOutput"
    ).ap()

    # x viewed as [b, dd, cb, h, w] where channel c = cb*64 + dd, cb = 2i+k.
    xr = x.rearrange("b (cb dd) h w -> b dd cb h w", cb=4)

    def body(inp, outp):
        # b=0 reads (partitions 0-63, even SBUF ports) on the sync ring,
        # b=1 reads (partitions 64-127, odd ports) on the gpsimd ring:
        # disjoint port sets per ring, no cross-ring port contention.
        rengs = [nc.sync, nc.gpsimd]
        for t in range(NT):
            h0 = t * HT
            it = inp.tile([128, 4 * HT * W], mybir.dt.float32)
            # one read DMA per (b, cb): clean 2-dim APs [[16384,64],[1,2048]]
            for b in range(2):
                for cb in range(4):
                    rengs[b].dma_start(
                        it[
                            b * 64 : (b + 1) * 64,
                            cb * HT * W : (cb + 1) * HT * W,
                        ],
                        xr[b, :, cb, h0 : h0 + HT, :],
                    )

            ot = outp.tile([128, HT * 4 * W], mybir.dt.float32)
            # ot[p, h*4W + i*2W + w*2 + k] = it[p, (2i+k)*HT*W + h*W + w]
            sv = it[:].rearrange(
                "p (i k h w) -> p i h w k", i=2, k=2, h=HT, w=W
            )
            dv = ot[:].rearrange(
                "p (h i w k) -> p i h w k", h=HT, i=2, w=W, k=2
            )
            for i in range(2):
                nc.vector.tensor_copy(dv[:, i], sv[:, i])

            nc.scalar.dma_start(out[:, :, 2 * h0 : 2 * h0 + 2 * HT, :], ot[:])

    with tile.TileContext(nc) as tc:
        with tc.tile_pool(name="inp", bufs=3) as inp, tc.tile_pool(
            name="outp", bufs=3
        ) as outp:
            if reps == 1:
                body(inp, outp)
            else:
                with tc.For_i(0, reps, 1):
                    body(inp, outp)
    nc.compile()
    return nc


def kernel(x: np.ndarray) -> np.ndarray:
    global _cached_nc
    if _cached_nc is None:
        _cached_nc = _build()
    nc = _cached_nc

    x = np.ascontiguousarray(x, dtype=np.float32)
    in_maps = [
        {"x": np.ascontiguousarray(x[c * BL : (c + 1) * BL])} for c in range(NCORES)
    ]
    res = bass_utils.run_bass_kernel_spmd(nc, in_maps, core_ids=list(range(NCORES)))
    return np.concatenate([r["out"] for r in res.results], axis=0)

